# revision 9
# baseline (speedup 1.0000x reference)
"""Trainium2 Bass kernel: multi-head causal attention (B=2, T=2048, C=1024, H=16).

Sharding: 8 cores = data parallel over B (2) x tensor parallel over head
groups (4 groups of 4 heads).  Each core computes its batch's partial
output contribution from its 4 heads through Wo rows; the host sums the 4
partials per batch (the "all-reduce") and adds the folded biases.

Device pipeline (per core, 4 heads; matmul operands bf16, PSUM fp32):
  - Q/K/V arrive HOST-pre-transposed as [C, T] bf16, so projections need no
    on-chip transposes: qT/kT = Wq^T @ X^T laid out [head_dim, T] directly,
    v natural [T, dv] with an extra always-1.0 65th column per head.
  - scores are computed TRANSPOSED: scT[k, q] = kT_blk^T @ qT_chunk, one
    512-wide PE matmul per (key-block, query-chunk).  The causal mask on the
    diagonal block is added by a second accumulating PE matmul
    (triT^T @ I = -1e9 strict-lower in [k,q] coords) - no vector-engine work.
  - one Exp (scale=1/8) per PSUM pair-strip writes expT[k, q] bf16; no
    normalization yet.
  - attn@v: out[q, dv+1] accumulates expT_blk^T @ [v|1] over key blocks; the
    65th column is the softmax row-sum for free.  out = out[:, :64] * (1/sum)
    via one broadcast DVE multiply per (chunk, head).
  - per chunk: PE-transpose out -> outT[dims, q]; output projection
    fin[q, C] = outT^T @ Wo streams wide; DMA fin to DRAM fp32.
"""

from contextlib import ExitStack

import numpy as np
import ml_dtypes

import concourse.bass as bass
import concourse.mybir as mybir
import concourse.tile as tile
from concourse import bacc
from concourse.bass_utils import run_bass_kernel_spmd

B, T, C = 2, 2048, 1024
H, DK, DV = 16, 64, 64
N_CORES = 8
GROUPS = 4                 # head groups (tensor parallel)
HPG = H // GROUPS          # 4 heads per group
GD = HPG * DK              # 256 head dims per group
P = 128
TCH = 512                  # query chunk for attention
NCB = C // P               # contraction chunks over C

BF = mybir.dt.bfloat16
F32 = mybir.dt.float32
AX = mybir.AxisListType
AF = mybir.ActivationFunctionType

bf16 = ml_dtypes.bfloat16

CFG = {"xin_bufs": 2, "sc_bufs": 2, "mm_bufs": 2, "out4_bufs": 2,
       "expt_bufs": 3, "fin_bufs": 3, "osb_bufs": 2,
       # ablation knobs (A/B attribution only; break correctness when off)
       "do_dma_in": True, "do_scores": True, "do_exp": True, "do_attnv": True,
       "do_wo": True, "do_fin": True, "interleave": False}


def _emit(nc, tc, io, t_len, ctx):
    NT = t_len // P            # 128-blocks of T
    NQC = t_len // TCH         # 512-chunks of T

    cpool = ctx.enter_context(tc.tile_pool(name="const", bufs=1))
    spool = ctx.enter_context(tc.tile_pool(name="stream", bufs=2))
    ppool = ctx.enter_context(tc.tile_pool(name="pers", bufs=1))
    apool = ctx.enter_context(tc.tile_pool(name="attn", bufs=2))
    pp = ctx.enter_context(tc.tile_pool(name="ps", bufs=2, space="PSUM"))

    # ---- constants / weights ------------------------------------------------
    ident = cpool.tile([P, P], BF)
    nc.sync.dma_start(out=ident, in_=io["ident"][:, :])
    triT = cpool.tile([P, P], BF)     # -1e9 strict upper (row q, col k)
    nc.sync.dma_start(out=triT, in_=io["triT"][:, :])
    bq_sb = cpool.tile([P, 2], F32)
    nc.sync.dma_start(out=bq_sb, in_=io["bq"][:, :])
    bk_sb = cpool.tile([P, 2], F32)
    nc.sync.dma_start(out=bk_sb, in_=io["bk"][:, :])

    wq_sb = cpool.tile([P, NCB, GD], BF)
    wk_sb = cpool.tile([P, NCB, GD], BF)
    wv_sb = cpool.tile([P, NCB, GD], BF)
    for w_sb, name in ((wq_sb, "wq"), (wk_sb, "wk"), (wv_sb, "wv")):
        nc.sync.dma_start(
            out=w_sb,
            in_=io[name][:, :].rearrange("(c p) d -> p c d", p=P))
    wo_sb = cpool.tile([P, 2, C], BF)
    nc.sync.dma_start(
        out=wo_sb, in_=io["wo"][:, :].rearrange("(r p) d -> p r d", p=P))

    # persistent activations
    qT_sb = ppool.tile([P, 2, t_len], BF)    # [pair head dims(128), pair, T]
    kT_sb = ppool.tile([P, 2, t_len], BF)
    v65_sb = ppool.tile([P, NT, HPG, DV + 1], BF)  # [T(k) % P, kblock, head, dv|1]
    outT_sb = ppool.tile([P, 2, t_len], BF)  # [pair head dims, pair, T]

    # ones column for the attn row-sum trick
    nc.gpsimd.memset(v65_sb[:, :, :, DV:DV + 1], 1.0)

    # ---- stage 1: projections from host-pre-transposed inputs --------------
    def load_project(t4):
        t0 = t4 * TCH
        xq = spool.tile([P, NCB, TCH], BF, tag="xq", bufs=CFG["xin_bufs"])
        xk = spool.tile([P, NCB, TCH], BF, tag="xk", bufs=CFG["xin_bufs"])
        xv = spool.tile([P, NCB, TCH], BF, tag="xv", bufs=CFG["xin_bufs"])
        if CFG["do_dma_in"]:
            for x_sb, name in ((xq, "qT"), (xk, "kT"), (xv, "vT")):
                nc.sync.dma_start(
                    out=x_sb,
                    in_=io[name][:, t0:t0 + TCH].rearrange(
                        "(c p) t -> p c t", p=P))
        for x_sb, w_sb, bias_sb, xT_sb in ((xq, wq_sb, bq_sb, qT_sb),
                                           (xk, wk_sb, bk_sb, kT_sb)):
            for pr in range(2):
                ps = pp.tile([P, TCH], F32, tag="mm", bufs=CFG["mm_bufs"])
                for cb in range(NCB):
                    nc.tensor.matmul(
                        ps, w_sb[:, cb, pr * P:(pr + 1) * P], x_sb[:, cb, :],
                        start=(cb == 0), stop=(cb == NCB - 1))
                nc.vector.tensor_scalar_add(
                    xT_sb[:, pr, t0:t0 + TCH], ps, bias_sb[:, pr:pr + 1])
        for tb in range(TCH // P):
            ps = pp.tile([P, TCH], F32, tag="mm", bufs=CFG["mm_bufs"])
            for cb in range(NCB):
                nc.tensor.matmul(
                    ps[:, :GD], xv[:, cb, tb * P:(tb + 1) * P], wv_sb[:, cb, :],
                    start=(cb == 0), stop=(cb == NCB - 1))
            nc.vector.tensor_copy(
                v65_sb[:, t4 * (TCH // P) + tb, :, 0:DV],
                ps[:, :GD].rearrange("p (h d) -> p h d", h=HPG))

    # ---- stage 2: attention per (query chunk, head) -------------------------
    def attend(qc, h, out_sb):
        pr, hs = h // 2, (h % 2) * DK
        nkb = (qc + 1) * (TCH // P)     # causal key 128-blocks
        expT = apool.tile([P, NT * TCH], BF, tag="expT", bufs=CFG["expt_bufs"])
        if not (CFG["do_scores"] and CFG["do_exp"]):
            nc.gpsimd.memset(expT[:, 0:P], 0.01)
        for kb0 in range(0, nkb, 2) if CFG["do_scores"] else ():
            sc = pp.tile([P, 2 * TCH], F32, tag="sc", bufs=CFG["sc_bufs"])
            for j in range(2):
                kb = kb0 + j
                nc.tensor.matmul(
                    sc[:, j * TCH:(j + 1) * TCH],
                    kT_sb[hs:hs + DK, pr, kb * P:(kb + 1) * P],
                    qT_sb[hs:hs + DK, pr, qc * TCH:(qc + 1) * TCH],
                    start=True, stop=(kb < qc * 4))
                if kb >= qc * 4:
                    d = kb - qc * 4
                    nc.tensor.matmul(
                        sc[:, j * TCH + d * P:j * TCH + (d + 1) * P],
                        triT, ident, start=False, stop=True)
            if CFG["do_exp"]:
                off = max(0, (kb0 - qc * 4)) * P
                nc.scalar.activation(
                    expT[:, kb0 * TCH + off:(kb0 + 2) * TCH], sc[:, off:],
                    AF.Exp, scale=0.125)
        if not CFG["do_attnv"]:
            nc.gpsimd.memset(out_sb[:, :, h, :], 0.5)
            return
        out4 = pp.tile([P, 4, DV + 1], F32, tag="out4", bufs=CFG["out4_bufs"])
        for qs in range(4):
            qi = qc * 4 + qs
            for kb in range(qi + 1):
                nc.tensor.matmul(
                    out4[:, qs, :],
                    expT[:, kb * TCH + qs * P:kb * TCH + (qs + 1) * P],
                    v65_sb[:, kb, h, :],
                    start=(kb == 0), stop=(kb == qi))
        R = apool.tile([P, 4], F32, tag="R", bufs=4)
        nc.vector.reciprocal(R, out4[:, :, DV])
        nc.vector.tensor_mul(
            out_sb[:, :, h, :], out4[:, :, 0:DV],
            R.unsqueeze(2).broadcast_to((P, 4, DV)))

    # ---- stage 3: transpose + output projection per query chunk -------------
    def wo_project(qc, out_sb):
        if not CFG["do_wo"]:
            return
        for pr in range(2):
            trp = pp.tile([P, 2 * TCH], BF, tag="mm", bufs=CFG["mm_bufs"])
            for qs in range(4):
                nc.tensor.transpose(
                    trp[:, qs * P:(qs + 1) * P],
                    out_sb[:, qs, 2 * pr:2 * pr + 2, :], ident)
            nc.vector.tensor_copy(
                outT_sb[:, pr, qc * TCH:(qc + 1) * TCH], trp[:, 0:TCH])
        for tb in range(qc * 4, qc * 4 + 4):
            fin = spool.tile([P, C], F32, tag="fin", bufs=CFG["fin_bufs"])
            for cc in range(C // TCH):
                ps = pp.tile([P, TCH], F32, tag="mm", bufs=CFG["mm_bufs"])
                for pr in range(2):
                    nc.tensor.matmul(
                        ps, outT_sb[:, pr, tb * P:(tb + 1) * P],
                        wo_sb[:, pr, cc * TCH:(cc + 1) * TCH],
                        start=(pr == 0), stop=(pr == 1))
                if CFG["do_fin"]:
                    nc.vector.tensor_copy(fin[:, cc * TCH:(cc + 1) * TCH], ps)
            if CFG["do_fin"]:
                nc.sync.dma_start(out=io["out"][tb * P:(tb + 1) * P, :], in_=fin)

    def attend_chunk(qc):
        out_sb = apool.tile([P, 4, HPG, DV], BF, tag="osb", bufs=CFG["osb_bufs"])
        for h in range(HPG):
            attend(qc, h, out_sb)
        wo_project(qc, out_sb)

    if CFG["interleave"]:
        load_project(0)
        load_project(1)
        attend_chunk(0)
        load_project(2)
        attend_chunk(1)
        load_project(3)
        for qc in range(2, NQC):
            attend_chunk(qc)
    else:
        for t4 in range(NQC):
            load_project(t4)
        for qc in range(NQC):
            attend_chunk(qc)


def _build(t_len=T, reps=1):
    nc = bacc.Bacc("TRN2", target_bir_lowering=False, debug=False,
                   num_devices=N_CORES)
    io = {
        "qT": nc.dram_tensor("qT", [C, t_len], BF, kind="ExternalInput"),
        "kT": nc.dram_tensor("kT", [C, t_len], BF, kind="ExternalInput"),
        "vT": nc.dram_tensor("vT", [C, t_len], BF, kind="ExternalInput"),
        "wq": nc.dram_tensor("wq", [C, GD], BF, kind="ExternalInput"),
        "wk": nc.dram_tensor("wk", [C, GD], BF, kind="ExternalInput"),
        "wv": nc.dram_tensor("wv", [C, GD], BF, kind="ExternalInput"),
        "wo": nc.dram_tensor("wo", [GD, C], BF, kind="ExternalInput"),
        "bq": nc.dram_tensor("bq", [P, 2], F32, kind="ExternalInput"),
        "bk": nc.dram_tensor("bk", [P, 2], F32, kind="ExternalInput"),
        "ident": nc.dram_tensor("ident", [P, P], BF, kind="ExternalInput"),
        "triT": nc.dram_tensor("triT", [P, P], BF, kind="ExternalInput"),
        "out": nc.dram_tensor("out", [t_len, C], F32, kind="ExternalOutput"),
    }
    with tile.TileContext(nc) as tc, ExitStack() as ctx:
        if reps == 1:
            _emit(nc, tc, io, t_len, ctx)
        else:
            hints = (mybir.EngineType.PE, mybir.EngineType.DVE,
                     mybir.EngineType.Activation, mybir.EngineType.Pool,
                     mybir.EngineType.SP)
            with tc.For_i(0, reps, 1, hint_engines=hints):
                _emit(nc, tc, io, t_len, ctx)
    nc.compile()
    return nc


_NC_CACHE = {}


def _get_nc(t_len=T, reps=1):
    key = (t_len, reps, tuple(sorted(CFG.items())))
    if key not in _NC_CACHE:
        _NC_CACHE[key] = _build(t_len, reps)
    return _NC_CACHE[key]


def _host_constants():
    ident = np.eye(P, dtype=bf16)
    triT = np.triu(np.full((P, P), -1e9, np.float32), 1).astype(bf16)
    return ident, triT


def make_in_maps(inputs, t_len=T):
    Q, K, V = inputs["Q"], inputs["K"], inputs["V"]
    Wq, bq = inputs["Wq"], inputs["bq"]
    Wk, bk = inputs["Wk"], inputs["bk"]
    Wv = inputs["Wv"]
    Wo = inputs["Wo"]
    ident, triT = _host_constants()
    qTs = [np.ascontiguousarray(Q[b, :t_len].T).astype(bf16) for b in range(B)]
    kTs = [np.ascontiguousarray(K[b, :t_len].T).astype(bf16) for b in range(B)]
    vTs = [np.ascontiguousarray(V[b, :t_len].T).astype(bf16) for b in range(B)]
    in_maps = []
    for core in range(N_CORES):
        b, g = divmod(core, GROUPS)
        cs = slice(g * GD, (g + 1) * GD)
        in_maps.append({
            "qT": qTs[b],
            "kT": kTs[b],
            "vT": vTs[b],
            "wq": np.ascontiguousarray(Wq[:, cs]).astype(bf16),
            "wk": np.ascontiguousarray(Wk[:, cs]).astype(bf16),
            "wv": np.ascontiguousarray(Wv[:, cs]).astype(bf16),
            "wo": np.ascontiguousarray(Wo[cs, :]).astype(bf16),
            "bq": np.ascontiguousarray(bq[cs].reshape(2, P).T).astype(np.float32),
            "bk": np.ascontiguousarray(bk[cs].reshape(2, P).T).astype(np.float32),
            "ident": ident,
            "triT": triT,
        })
    return in_maps


def combine(results, inputs, t_len=T):
    bo, bv, Wo = inputs["bo"], inputs["bv"], inputs["Wo"]
    bias = (bo.astype(np.float64) + bv.astype(np.float64) @ Wo.astype(np.float64))
    out = np.empty((B, t_len, C), np.float32)
    for b in range(B):
        acc = np.zeros((t_len, C), np.float64)
        for g in range(GROUPS):
            acc += results[b * GROUPS + g]["out"].astype(np.float64)
        out[b] = (acc + bias).astype(np.float32)
    return out


def _mask_is_causal(mask, t_len):
    mask = np.asarray(mask)
    if mask.shape != (1, 1, t_len, t_len):
        return False
    m = mask[0, 0]
    tri = np.tril(np.ones((t_len, t_len), bool))
    return (m[tri] == 0.0).all() and (m[~tri] <= -1e8).all()


def _reference_fallback(inputs):
    # generic-mask fallback (never hit with the causal reference mask)
    Q, K, V = (np.asarray(inputs[k], np.float32) for k in ("Q", "K", "V"))
    mask = np.asarray(inputs["mask"], np.float32)
    out = np.empty((B, T, C), np.float32)
    for b in range(B):
        acc = np.zeros((T, C), np.float32)
        for h in range(H):
            q = Q[b] @ inputs["Wq"][:, h * DK:(h + 1) * DK] + inputs["bq"][h * DK:(h + 1) * DK]
            k = K[b] @ inputs["Wk"][:, h * DK:(h + 1) * DK] + inputs["bk"][h * DK:(h + 1) * DK]
            v = V[b] @ inputs["Wv"][:, h * DV:(h + 1) * DV] + inputs["bv"][h * DV:(h + 1) * DV]
            m = mask[min(b, mask.shape[0] - 1), min(h, mask.shape[1] - 1)]
            s = (q @ k.T + m) / np.sqrt(DK).astype(np.float32)
            s -= s.max(-1, keepdims=True)
            e = np.exp(s)
            a = e / e.sum(-1, keepdims=True)
            acc += (a @ v) @ inputs["Wo"][h * DV:(h + 1) * DV, :]
        out[b] = acc + inputs["bo"]
    return out


def kernel(**inputs):
    inputs = {k: np.asarray(v) for k, v in inputs.items()}
    if not _mask_is_causal(inputs["mask"], T):
        return _reference_fallback(inputs)
    nc = _get_nc(T)
    in_maps = make_in_maps(inputs, T)
    res = run_bass_kernel_spmd(nc, in_maps, core_ids=list(range(N_CORES)))
    return combine(res.results, inputs, T)


# revision 11
# speedup vs baseline: 1.0297x; 1.0297x over previous
"""Trainium2 Bass kernel: multi-head causal attention (B=2, T=2048, C=1024, H=16).

Sharding: 8 cores = data parallel over B (2) x tensor parallel over head
groups (4 groups of 4 heads).  Each core computes its batch's partial
output contribution from its 4 heads through Wo rows; the host sums the 4
partials per batch (the "all-reduce") and adds the folded biases.

Device pipeline (per core, 4 heads; matmul operands bf16, PSUM fp32):
  - Q/K/V arrive HOST-pre-transposed as [C, T] bf16, so projections need no
    on-chip transposes: qT/kT = Wq^T @ X^T laid out [head_dim, T] directly,
    v natural [T, dv] with an extra always-1.0 65th column per head.
  - scores are computed TRANSPOSED: scT[k, q] = kT_blk^T @ qT_chunk, one
    512-wide PE matmul per (key-block, query-chunk).  The causal mask on the
    diagonal block is added by a second accumulating PE matmul
    (triT^T @ I = -1e9 strict-lower in [k,q] coords) - no vector-engine work.
  - one Exp (scale=1/8) per PSUM pair-strip writes expT[k, q] bf16; no
    normalization yet.
  - attn@v: out[q, dv+1] accumulates expT_blk^T @ [v|1] over key blocks; the
    65th column is the softmax row-sum for free.  out = out[:, :64] * (1/sum)
    via one broadcast DVE multiply per (chunk, head).
  - per chunk: PE-transpose out -> outT[dims, q]; output projection
    fin[q, C] = outT^T @ Wo streams wide; DMA fin to DRAM fp32.
"""

from contextlib import ExitStack

import numpy as np
import ml_dtypes

import concourse.bass as bass
import concourse.mybir as mybir
import concourse.tile as tile
from concourse import bacc
from concourse.bass_utils import run_bass_kernel_spmd

B, T, C = 2, 2048, 1024
H, DK, DV = 16, 64, 64
N_CORES = 8
GROUPS = 4                 # head groups (tensor parallel)
HPG = H // GROUPS          # 4 heads per group
GD = HPG * DK              # 256 head dims per group
P = 128
TCH = 512                  # query chunk for attention
NCB = C // P               # contraction chunks over C

BF = mybir.dt.bfloat16
F32 = mybir.dt.float32
AX = mybir.AxisListType
AF = mybir.ActivationFunctionType

bf16 = ml_dtypes.bfloat16

CFG = {"xin_bufs": 2, "sc_bufs": 2, "mm_bufs": 2, "out4_bufs": 2,
       "expt_bufs": 2, "fin_bufs": 3, "osb_bufs": 2,
       # ablation knobs (A/B attribution only; break correctness when off)
       "do_dma_in": True, "do_scores": True, "do_exp": True, "do_attnv": True,
       "do_wo": True, "do_fin": True, "interleave": False, "pool_mask": True}


def _emit(nc, tc, io, t_len, ctx):
    NT = t_len // P            # 128-blocks of T
    NQC = t_len // TCH         # 512-chunks of T

    cpool = ctx.enter_context(tc.tile_pool(name="const", bufs=1))
    spool = ctx.enter_context(tc.tile_pool(name="stream", bufs=2))
    ppool = ctx.enter_context(tc.tile_pool(name="pers", bufs=1))
    apool = ctx.enter_context(tc.tile_pool(name="attn", bufs=2))
    pp = ctx.enter_context(tc.tile_pool(name="ps", bufs=2, space="PSUM"))

    # ---- constants / weights ------------------------------------------------
    ident = cpool.tile([P, P], BF)
    nc.sync.dma_start(out=ident, in_=io["ident"][:, :])
    triT = cpool.tile([P, P], BF)     # -1e9 strict upper (row q, col k)
    nc.sync.dma_start(out=triT, in_=io["triT"][:, :])
    bq_sb = cpool.tile([P, 2], F32)
    nc.sync.dma_start(out=bq_sb, in_=io["bq"][:, :])
    bk_sb = cpool.tile([P, 2], F32)
    nc.sync.dma_start(out=bk_sb, in_=io["bk"][:, :])

    wq_sb = cpool.tile([P, NCB, GD], BF)
    wk_sb = cpool.tile([P, NCB, GD], BF)
    wv_sb = cpool.tile([P, NCB, GD], BF)
    for w_sb, name in ((wq_sb, "wq"), (wk_sb, "wk"), (wv_sb, "wv")):
        nc.sync.dma_start(
            out=w_sb,
            in_=io[name][:, :].rearrange("(c p) d -> p c d", p=P))
    wo_sb = cpool.tile([P, 2, C], BF)
    nc.sync.dma_start(
        out=wo_sb, in_=io["wo"][:, :].rearrange("(r p) d -> p r d", p=P))

    # persistent activations
    qT_sb = ppool.tile([P, 2, t_len], BF)    # [pair head dims(128), pair, T]
    kT_sb = ppool.tile([P, 2, t_len], BF)
    v65_sb = ppool.tile([P, NT, HPG, DV + 1], BF)  # [T(k) % P, kblock, head, dv|1]
    outT_sb = ppool.tile([P, 2, t_len], BF)  # [pair head dims, pair, T]

    # ones column for the attn row-sum trick
    nc.gpsimd.memset(v65_sb[:, :, :, DV:DV + 1], 1.0)

    # ---- stage 1: projections from host-pre-transposed inputs --------------
    def load_project(t4):
        t0 = t4 * TCH
        xq = spool.tile([P, NCB, TCH], BF, tag="xq", bufs=CFG["xin_bufs"])
        xk = spool.tile([P, NCB, TCH], BF, tag="xk", bufs=CFG["xin_bufs"])
        xv = spool.tile([P, NCB, TCH], BF, tag="xv", bufs=CFG["xin_bufs"])
        if CFG["do_dma_in"]:
            for x_sb, name in ((xq, "qT"), (xk, "kT"), (xv, "vT")):
                nc.sync.dma_start(
                    out=x_sb,
                    in_=io[name][:, t0:t0 + TCH].rearrange(
                        "(c p) t -> p c t", p=P))
        for x_sb, w_sb, bias_sb, xT_sb in ((xq, wq_sb, bq_sb, qT_sb),
                                           (xk, wk_sb, bk_sb, kT_sb)):
            for pr in range(2):
                ps = pp.tile([P, TCH], F32, tag="mm", bufs=CFG["mm_bufs"])
                for cb in range(NCB):
                    nc.tensor.matmul(
                        ps, w_sb[:, cb, pr * P:(pr + 1) * P], x_sb[:, cb, :],
                        start=(cb == 0), stop=(cb == NCB - 1))
                nc.vector.tensor_scalar_add(
                    xT_sb[:, pr, t0:t0 + TCH], ps, bias_sb[:, pr:pr + 1])
        for tb in range(TCH // P):
            ps = pp.tile([P, TCH], F32, tag="mm", bufs=CFG["mm_bufs"])
            for cb in range(NCB):
                nc.tensor.matmul(
                    ps[:, :GD], xv[:, cb, tb * P:(tb + 1) * P], wv_sb[:, cb, :],
                    start=(cb == 0), stop=(cb == NCB - 1))
            nc.vector.tensor_copy(
                v65_sb[:, t4 * (TCH // P) + tb, :, 0:DV],
                ps[:, :GD].rearrange("p (h d) -> p h d", h=HPG))

    # ---- stage 2: attention per (query chunk, head) -------------------------
    def attend(qc, h, out_sb):
        pr, hs = h // 2, (h % 2) * DK
        nkb = (qc + 1) * (TCH // P)     # causal key 128-blocks
        expT = apool.tile([P, NT * TCH], BF, tag="expT", bufs=CFG["expt_bufs"])
        if not (CFG["do_scores"] and CFG["do_exp"]):
            nc.gpsimd.memset(expT[:, 0:P], 0.01)
        for kb0 in range(0, nkb, 2) if CFG["do_scores"] else ():
            sc = pp.tile([P, 2 * TCH], F32, tag="sc", bufs=CFG["sc_bufs"])
            for j in range(2):
                kb = kb0 + j
                d = kb - qc * 4
                off = max(0, d) * P
                nc.tensor.matmul(
                    sc[:, j * TCH + off:(j + 1) * TCH],
                    kT_sb[hs:hs + DK, pr, kb * P:(kb + 1) * P],
                    qT_sb[hs:hs + DK, pr, qc * TCH + off:(qc + 1) * TCH],
                    start=True, stop=(d < 0 or CFG["pool_mask"]))
                if d >= 0 and not CFG["pool_mask"]:
                    nc.tensor.matmul(
                        sc[:, j * TCH + d * P:j * TCH + (d + 1) * P],
                        triT, ident, start=False, stop=True)
            if CFG["do_exp"]:
                if kb0 >= qc * 4:
                    # diagonal pair: one exp per strip over its written range
                    for j in range(2):
                        kb = kb0 + j
                        d = kb - qc * 4
                        nc.scalar.activation(
                            expT[:, kb * TCH + d * P:(kb + 1) * TCH],
                            sc[:, j * TCH + d * P:(j + 1) * TCH],
                            AF.Exp, scale=0.125)
                else:
                    nc.scalar.activation(
                        expT[:, kb0 * TCH:(kb0 + 2) * TCH], sc,
                        AF.Exp, scale=0.125)
            if CFG["pool_mask"] and kb0 >= qc * 4 and CFG["do_exp"]:
                # zero the strict-lower (q < k) part of each diagonal block on
                # the idle GPSIMD engine instead of a PE mask matmul
                for j in range(2):
                    kb = kb0 + j
                    d = kb - qc * 4
                    blk = expT[:, kb * TCH + d * P:kb * TCH + (d + 1) * P]
                    nc.gpsimd.tensor_mul(blk, blk, triT)
        if not CFG["do_attnv"]:
            nc.gpsimd.memset(out_sb[:, :, h, :], 0.5)
            return
        out4 = pp.tile([P, 4, DV + 1], F32, tag="out4", bufs=CFG["out4_bufs"])
        for qs in range(4):
            qi = qc * 4 + qs
            for kb in range(qi + 1):
                nc.tensor.matmul(
                    out4[:, qs, :],
                    expT[:, kb * TCH + qs * P:kb * TCH + (qs + 1) * P],
                    v65_sb[:, kb, h, :],
                    start=(kb == 0), stop=(kb == qi))
        R = apool.tile([P, 4], F32, tag="R", bufs=4)
        nc.vector.reciprocal(R, out4[:, :, DV])
        nc.vector.tensor_mul(
            out_sb[:, :, h, :], out4[:, :, 0:DV],
            R.unsqueeze(2).broadcast_to((P, 4, DV)))

    # ---- stage 3: transpose + output projection per query chunk -------------
    def wo_project(qc, out_sb):
        if not CFG["do_wo"]:
            return
        for pr in range(2):
            trp = pp.tile([P, 2 * TCH], BF, tag="mm", bufs=CFG["mm_bufs"])
            for qs in range(4):
                nc.tensor.transpose(
                    trp[:, qs * P:(qs + 1) * P],
                    out_sb[:, qs, 2 * pr:2 * pr + 2, :], ident)
            nc.vector.tensor_copy(
                outT_sb[:, pr, qc * TCH:(qc + 1) * TCH], trp[:, 0:TCH])
        for tb in range(qc * 4, qc * 4 + 4):
            fin = spool.tile([P, C], F32, tag="fin", bufs=CFG["fin_bufs"])
            for cc in range(C // TCH):
                ps = pp.tile([P, TCH], F32, tag="mm", bufs=CFG["mm_bufs"])
                for pr in range(2):
                    nc.tensor.matmul(
                        ps, outT_sb[:, pr, tb * P:(tb + 1) * P],
                        wo_sb[:, pr, cc * TCH:(cc + 1) * TCH],
                        start=(pr == 0), stop=(pr == 1))
                if CFG["do_fin"]:
                    nc.vector.tensor_copy(fin[:, cc * TCH:(cc + 1) * TCH], ps)
            if CFG["do_fin"]:
                nc.sync.dma_start(out=io["out"][tb * P:(tb + 1) * P, :], in_=fin)

    def attend_chunk(qc):
        out_sb = apool.tile([P, 4, HPG, DV], BF, tag="osb", bufs=CFG["osb_bufs"])
        for h in range(HPG):
            attend(qc, h, out_sb)
        wo_project(qc, out_sb)

    if CFG["interleave"]:
        load_project(0)
        load_project(1)
        attend_chunk(0)
        load_project(2)
        attend_chunk(1)
        load_project(3)
        for qc in range(2, NQC):
            attend_chunk(qc)
    else:
        for t4 in range(NQC):
            load_project(t4)
        for qc in range(NQC):
            attend_chunk(qc)


def _build(t_len=T, reps=1):
    nc = bacc.Bacc("TRN2", target_bir_lowering=False, debug=False,
                   num_devices=N_CORES)
    io = {
        "qT": nc.dram_tensor("qT", [C, t_len], BF, kind="ExternalInput"),
        "kT": nc.dram_tensor("kT", [C, t_len], BF, kind="ExternalInput"),
        "vT": nc.dram_tensor("vT", [C, t_len], BF, kind="ExternalInput"),
        "wq": nc.dram_tensor("wq", [C, GD], BF, kind="ExternalInput"),
        "wk": nc.dram_tensor("wk", [C, GD], BF, kind="ExternalInput"),
        "wv": nc.dram_tensor("wv", [C, GD], BF, kind="ExternalInput"),
        "wo": nc.dram_tensor("wo", [GD, C], BF, kind="ExternalInput"),
        "bq": nc.dram_tensor("bq", [P, 2], F32, kind="ExternalInput"),
        "bk": nc.dram_tensor("bk", [P, 2], F32, kind="ExternalInput"),
        "ident": nc.dram_tensor("ident", [P, P], BF, kind="ExternalInput"),
        "triT": nc.dram_tensor("triT", [P, P], BF, kind="ExternalInput"),
        "out": nc.dram_tensor("out", [t_len, C], F32, kind="ExternalOutput"),
    }
    with tile.TileContext(nc) as tc, ExitStack() as ctx:
        if reps == 1:
            _emit(nc, tc, io, t_len, ctx)
        else:
            hints = (mybir.EngineType.PE, mybir.EngineType.DVE,
                     mybir.EngineType.Activation, mybir.EngineType.Pool,
                     mybir.EngineType.SP)
            with tc.For_i(0, reps, 1, hint_engines=hints):
                _emit(nc, tc, io, t_len, ctx)
    nc.compile()
    return nc


_NC_CACHE = {}


def _get_nc(t_len=T, reps=1):
    key = (t_len, reps, tuple(sorted(CFG.items())))
    if key not in _NC_CACHE:
        _NC_CACHE[key] = _build(t_len, reps)
    return _NC_CACHE[key]


def _host_constants():
    ident = np.eye(P, dtype=bf16)
    if CFG["pool_mask"]:
        # keep-mask: tri01[k, q] = 1 where q >= k (causal-valid), else 0
        triT = np.triu(np.ones((P, P), np.float32)).astype(bf16)
    else:
        triT = np.triu(np.full((P, P), -1e9, np.float32), 1).astype(bf16)
    return ident, triT


def make_in_maps(inputs, t_len=T):
    Q, K, V = inputs["Q"], inputs["K"], inputs["V"]
    Wq, bq = inputs["Wq"], inputs["bq"]
    Wk, bk = inputs["Wk"], inputs["bk"]
    Wv = inputs["Wv"]
    Wo = inputs["Wo"]
    ident, triT = _host_constants()
    qTs = [np.ascontiguousarray(Q[b, :t_len].T).astype(bf16) for b in range(B)]
    kTs = [np.ascontiguousarray(K[b, :t_len].T).astype(bf16) for b in range(B)]
    vTs = [np.ascontiguousarray(V[b, :t_len].T).astype(bf16) for b in range(B)]
    in_maps = []
    for core in range(N_CORES):
        b, g = divmod(core, GROUPS)
        cs = slice(g * GD, (g + 1) * GD)
        in_maps.append({
            "qT": qTs[b],
            "kT": kTs[b],
            "vT": vTs[b],
            "wq": np.ascontiguousarray(Wq[:, cs]).astype(bf16),
            "wk": np.ascontiguousarray(Wk[:, cs]).astype(bf16),
            "wv": np.ascontiguousarray(Wv[:, cs]).astype(bf16),
            "wo": np.ascontiguousarray(Wo[cs, :]).astype(bf16),
            "bq": np.ascontiguousarray(bq[cs].reshape(2, P).T).astype(np.float32),
            "bk": np.ascontiguousarray(bk[cs].reshape(2, P).T).astype(np.float32),
            "ident": ident,
            "triT": triT,
        })
    return in_maps


def combine(results, inputs, t_len=T):
    bo, bv, Wo = inputs["bo"], inputs["bv"], inputs["Wo"]
    bias = (bo.astype(np.float64) + bv.astype(np.float64) @ Wo.astype(np.float64))
    out = np.empty((B, t_len, C), np.float32)
    for b in range(B):
        acc = np.zeros((t_len, C), np.float64)
        for g in range(GROUPS):
            acc += results[b * GROUPS + g]["out"].astype(np.float64)
        out[b] = (acc + bias).astype(np.float32)
    return out


def _mask_is_causal(mask, t_len):
    mask = np.asarray(mask)
    if mask.shape != (1, 1, t_len, t_len):
        return False
    m = mask[0, 0]
    tri = np.tril(np.ones((t_len, t_len), bool))
    return (m[tri] == 0.0).all() and (m[~tri] <= -1e8).all()


def _reference_fallback(inputs):
    # generic-mask fallback (never hit with the causal reference mask)
    Q, K, V = (np.asarray(inputs[k], np.float32) for k in ("Q", "K", "V"))
    mask = np.asarray(inputs["mask"], np.float32)
    out = np.empty((B, T, C), np.float32)
    for b in range(B):
        acc = np.zeros((T, C), np.float32)
        for h in range(H):
            q = Q[b] @ inputs["Wq"][:, h * DK:(h + 1) * DK] + inputs["bq"][h * DK:(h + 1) * DK]
            k = K[b] @ inputs["Wk"][:, h * DK:(h + 1) * DK] + inputs["bk"][h * DK:(h + 1) * DK]
            v = V[b] @ inputs["Wv"][:, h * DV:(h + 1) * DV] + inputs["bv"][h * DV:(h + 1) * DV]
            m = mask[min(b, mask.shape[0] - 1), min(h, mask.shape[1] - 1)]
            s = (q @ k.T + m) / np.sqrt(DK).astype(np.float32)
            s -= s.max(-1, keepdims=True)
            e = np.exp(s)
            a = e / e.sum(-1, keepdims=True)
            acc += (a @ v) @ inputs["Wo"][h * DV:(h + 1) * DV, :]
        out[b] = acc + inputs["bo"]
    return out


def kernel(**inputs):
    inputs = {k: np.asarray(v) for k, v in inputs.items()}
    if not _mask_is_causal(inputs["mask"], T):
        return _reference_fallback(inputs)
    nc = _get_nc(T)
    in_maps = make_in_maps(inputs, T)
    res = run_bass_kernel_spmd(nc, in_maps, core_ids=list(range(N_CORES)))
    return combine(res.results, inputs, T)


# revision 12
# speedup vs baseline: 1.7146x; 1.6651x over previous
"""Trainium2 Bass kernel: multi-head causal attention (B=2, T=2048, C=1024, H=16).

Sharding: 8 cores = data parallel over B (2) x tensor parallel over head
groups (4 groups of 4 heads).  Each core computes its batch's partial
output contribution from its 4 heads through Wo rows; the host sums the 4
partials per batch (the "all-reduce") and adds the folded biases.

Device pipeline (per core, 4 heads; matmul operands bf16, PSUM fp32):
  - Q/K/V arrive HOST-pre-transposed as [C, T] bf16, so projections need no
    on-chip transposes: qT/kT = Wq^T @ X^T laid out [head_dim, T] directly,
    v natural [T, dv] with an extra always-1.0 65th column per head.
  - scores are computed TRANSPOSED: scT[k, q] = kT_blk^T @ qT_chunk, one
    512-wide PE matmul per (key-block, query-chunk).  The causal mask on the
    diagonal block is added by a second accumulating PE matmul
    (triT^T @ I = -1e9 strict-lower in [k,q] coords) - no vector-engine work.
  - one Exp (scale=1/8) per PSUM pair-strip writes expT[k, q] bf16; no
    normalization yet.
  - attn@v: out[q, dv+1] accumulates expT_blk^T @ [v|1] over key blocks; the
    65th column is the softmax row-sum for free.  out = out[:, :64] * (1/sum)
    via one broadcast DVE multiply per (chunk, head).
  - per chunk: PE-transpose out -> outT[dims, q]; output projection
    fin[q, C] = outT^T @ Wo streams wide; DMA fin to DRAM fp32.
"""

from contextlib import ExitStack

import numpy as np
import ml_dtypes

import concourse.bass as bass
import concourse.mybir as mybir
import concourse.tile as tile
from concourse import bacc
from concourse.bass_utils import run_bass_kernel_spmd

B, T, C = 2, 2048, 1024
H, DK, DV = 16, 64, 64
N_CORES = 8
GROUPS = 4                 # head groups (tensor parallel)
HPG = H // GROUPS          # 4 heads per group
GD = HPG * DK              # 256 head dims per group
P = 128
TCH = 512                  # query chunk for attention
NCB = C // P               # contraction chunks over C

BF = mybir.dt.bfloat16
F32 = mybir.dt.float32
AX = mybir.AxisListType
AF = mybir.ActivationFunctionType

bf16 = ml_dtypes.bfloat16

CFG = {"xin_bufs": 2, "sc_bufs": 2, "mm_bufs": 2, "out4_bufs": 2,
       "expt_bufs": 2, "fin_bufs": 3, "osb_bufs": 2,
       # ablation knobs (A/B attribution only; break correctness when off)
       "do_dma_in": True, "do_scores": True, "do_exp": True, "do_attnv": True,
       "do_wo": True, "do_fin": True, "interleave": False, "pool_mask": True}


def _emit(nc, tc, io, t_len, ctx):
    NT = t_len // P            # 128-blocks of T
    NQC = t_len // TCH         # 512-chunks of T

    cpool = ctx.enter_context(tc.tile_pool(name="const", bufs=1))
    spool = ctx.enter_context(tc.tile_pool(name="stream", bufs=2))
    ppool = ctx.enter_context(tc.tile_pool(name="pers", bufs=1))
    apool = ctx.enter_context(tc.tile_pool(name="attn", bufs=2))
    pp = ctx.enter_context(tc.tile_pool(name="ps", bufs=2, space="PSUM"))

    # ---- constants / weights ------------------------------------------------
    ident = cpool.tile([P, P], BF)
    nc.sync.dma_start(out=ident, in_=io["ident"][:, :])
    triT = cpool.tile([P, P], BF)     # -1e9 strict upper (row q, col k)
    nc.sync.dma_start(out=triT, in_=io["triT"][:, :])
    bq_sb = cpool.tile([P, 2], F32)
    nc.sync.dma_start(out=bq_sb, in_=io["bq"][:, :])
    bk_sb = cpool.tile([P, 2], F32)
    nc.sync.dma_start(out=bk_sb, in_=io["bk"][:, :])

    wq_sb = cpool.tile([P, NCB, GD], BF)
    wk_sb = cpool.tile([P, NCB, GD], BF)
    wv_sb = cpool.tile([P, NCB, GD], BF)
    for w_sb, name in ((wq_sb, "wq"), (wk_sb, "wk"), (wv_sb, "wv")):
        nc.sync.dma_start(
            out=w_sb,
            in_=io[name][:, :].rearrange("(c p) d -> p c d", p=P))
    wo_sb = cpool.tile([P, 2, C], BF)
    nc.sync.dma_start(
        out=wo_sb, in_=io["wo"][:, :].rearrange("(r p) d -> p r d", p=P))

    # persistent activations
    qT_sb = ppool.tile([P, 2, t_len], BF)    # [pair head dims(128), pair, T]
    kT_sb = ppool.tile([P, 2, t_len], BF)
    v65_sb = ppool.tile([P, NT, HPG, DV + 1], BF)  # [T(k) % P, kblock, head, dv|1]
    outT_sb = ppool.tile([P, 2, t_len], BF)  # [pair head dims, pair, T]

    # ones column for the attn row-sum trick
    nc.gpsimd.memset(v65_sb[:, :, :, DV:DV + 1], 1.0)

    # ---- stage 1: projections from host-pre-transposed inputs --------------
    def load_project(t4):
        t0 = t4 * TCH
        xq = spool.tile([P, NCB, TCH], BF, tag="xq", bufs=CFG["xin_bufs"])
        xk = spool.tile([P, NCB, TCH], BF, tag="xk", bufs=CFG["xin_bufs"])
        xv = spool.tile([P, NCB, TCH], BF, tag="xv", bufs=CFG["xin_bufs"])
        if CFG["do_dma_in"]:
            for x_sb, name in ((xq, "qT"), (xk, "kT"), (xv, "vT")):
                nc.sync.dma_start(
                    out=x_sb,
                    in_=io[name][:, t0:t0 + TCH].rearrange(
                        "(c p) t -> p c t", p=P))
        for x_sb, w_sb, bias_sb, xT_sb in ((xq, wq_sb, bq_sb, qT_sb),
                                           (xk, wk_sb, bk_sb, kT_sb)):
            for pr in range(2):
                ps = pp.tile([P, TCH], F32, tag="mm", bufs=CFG["mm_bufs"])
                for cb in range(NCB):
                    nc.tensor.matmul(
                        ps, w_sb[:, cb, pr * P:(pr + 1) * P], x_sb[:, cb, :],
                        start=(cb == 0), stop=(cb == NCB - 1))
                nc.vector.tensor_scalar_add(
                    xT_sb[:, pr, t0:t0 + TCH], ps, bias_sb[:, pr:pr + 1])
        for tb in range(TCH // P):
            ps = pp.tile([P, TCH], F32, tag="mm", bufs=CFG["mm_bufs"])
            for cb in range(NCB):
                nc.tensor.matmul(
                    ps[:, :GD], xv[:, cb, tb * P:(tb + 1) * P], wv_sb[:, cb, :],
                    start=(cb == 0), stop=(cb == NCB - 1))
            nc.vector.tensor_copy(
                v65_sb[:, t4 * (TCH // P) + tb, :, 0:DV],
                ps[:, :GD].rearrange("p (h d) -> p h d", h=HPG))

    # ---- stage 2: attention per (query chunk, head) -------------------------
    def scores_exp(qc, h):
        pr, hs = h // 2, (h % 2) * DK
        nkb = (qc + 1) * (TCH // P)     # causal key 128-blocks
        expT = apool.tile([P, NT * TCH], BF, tag="expT", bufs=CFG["expt_bufs"])
        if not (CFG["do_scores"] and CFG["do_exp"]):
            nc.gpsimd.memset(expT[:, 0:P], 0.01)
        for kb0 in range(0, nkb, 2) if CFG["do_scores"] else ():
            sc = pp.tile([P, 2 * TCH], F32, tag="sc", bufs=CFG["sc_bufs"])
            for j in range(2):
                kb = kb0 + j
                d = kb - qc * 4
                off = max(0, d) * P
                nc.tensor.matmul(
                    sc[:, j * TCH + off:(j + 1) * TCH],
                    kT_sb[hs:hs + DK, pr, kb * P:(kb + 1) * P],
                    qT_sb[hs:hs + DK, pr, qc * TCH + off:(qc + 1) * TCH],
                    start=True, stop=(d < 0 or CFG["pool_mask"]))
                if d >= 0 and not CFG["pool_mask"]:
                    nc.tensor.matmul(
                        sc[:, j * TCH + d * P:j * TCH + (d + 1) * P],
                        triT, ident, start=False, stop=True)
            if CFG["do_exp"]:
                if kb0 >= qc * 4:
                    # diagonal pair: one exp per strip over its written range
                    for j in range(2):
                        kb = kb0 + j
                        d = kb - qc * 4
                        nc.scalar.activation(
                            expT[:, kb * TCH + d * P:(kb + 1) * TCH],
                            sc[:, j * TCH + d * P:(j + 1) * TCH],
                            AF.Exp, scale=0.125)
                else:
                    nc.scalar.activation(
                        expT[:, kb0 * TCH:(kb0 + 2) * TCH], sc,
                        AF.Exp, scale=0.125)
            if CFG["pool_mask"] and kb0 >= qc * 4 and CFG["do_exp"]:
                # zero the strict-lower (q < k) part of each diagonal block on
                # the idle GPSIMD engine instead of a PE mask matmul
                for j in range(2):
                    kb = kb0 + j
                    d = kb - qc * 4
                    blk = expT[:, kb * TCH + d * P:kb * TCH + (d + 1) * P]
                    nc.gpsimd.tensor_mul(blk, blk, triT)
        return expT

    def attnv(qc, h, expT, out_sb):
        if not CFG["do_attnv"]:
            nc.gpsimd.memset(out_sb[:, :, h, :], 0.5)
            return
        out4 = pp.tile([P, 4, DV + 1], F32, tag="out4", bufs=CFG["out4_bufs"])
        for qs in range(4):
            qi = qc * 4 + qs
            for kb in range(qi + 1):
                nc.tensor.matmul(
                    out4[:, qs, :],
                    expT[:, kb * TCH + qs * P:kb * TCH + (qs + 1) * P],
                    v65_sb[:, kb, h, :],
                    start=(kb == 0), stop=(kb == qi))
        R = apool.tile([P, 4], F32, tag="R", bufs=4)
        nc.vector.reciprocal(R, out4[:, :, DV])
        nc.vector.tensor_mul(
            out_sb[:, :, h, :], out4[:, :, 0:DV],
            R.unsqueeze(2).broadcast_to((P, 4, DV)))

    # ---- stage 3: transpose + output projection per query chunk -------------
    def wo_project(qc, out_sb):
        if not CFG["do_wo"]:
            return
        for pr in range(2):
            trp = pp.tile([P, 2 * TCH], BF, tag="mm", bufs=CFG["mm_bufs"])
            for qs in range(4):
                nc.tensor.transpose(
                    trp[:, qs * P:(qs + 1) * P],
                    out_sb[:, qs, 2 * pr:2 * pr + 2, :], ident)
            nc.vector.tensor_copy(
                outT_sb[:, pr, qc * TCH:(qc + 1) * TCH], trp[:, 0:TCH])
        for tb in range(qc * 4, qc * 4 + 4):
            fin = spool.tile([P, C], F32, tag="fin", bufs=CFG["fin_bufs"])
            for cc in range(C // TCH):
                ps = pp.tile([P, TCH], F32, tag="mm", bufs=CFG["mm_bufs"])
                for pr in range(2):
                    nc.tensor.matmul(
                        ps, outT_sb[:, pr, tb * P:(tb + 1) * P],
                        wo_sb[:, pr, cc * TCH:(cc + 1) * TCH],
                        start=(pr == 0), stop=(pr == 1))
                if CFG["do_fin"]:
                    nc.vector.tensor_copy(fin[:, cc * TCH:(cc + 1) * TCH], ps)
            if CFG["do_fin"]:
                nc.sync.dma_start(out=io["out"][tb * P:(tb + 1) * P, :], in_=fin)

    def attend_chunk(qc):
        out_sb = apool.tile([P, 4, HPG, DV], BF, tag="osb", bufs=CFG["osb_bufs"])
        prev = None
        for h in range(HPG):
            expT = scores_exp(qc, h)
            if prev is not None:
                attnv(qc, prev[0], prev[1], out_sb)
            prev = (h, expT)
        attnv(qc, prev[0], prev[1], out_sb)
        wo_project(qc, out_sb)

    if CFG["interleave"]:
        load_project(0)
        load_project(1)
        attend_chunk(0)
        load_project(2)
        attend_chunk(1)
        load_project(3)
        for qc in range(2, NQC):
            attend_chunk(qc)
    else:
        for t4 in range(NQC):
            load_project(t4)
        for qc in range(NQC):
            attend_chunk(qc)


def _build(t_len=T, reps=1):
    nc = bacc.Bacc("TRN2", target_bir_lowering=False, debug=False,
                   num_devices=N_CORES)
    io = {
        "qT": nc.dram_tensor("qT", [C, t_len], BF, kind="ExternalInput"),
        "kT": nc.dram_tensor("kT", [C, t_len], BF, kind="ExternalInput"),
        "vT": nc.dram_tensor("vT", [C, t_len], BF, kind="ExternalInput"),
        "wq": nc.dram_tensor("wq", [C, GD], BF, kind="ExternalInput"),
        "wk": nc.dram_tensor("wk", [C, GD], BF, kind="ExternalInput"),
        "wv": nc.dram_tensor("wv", [C, GD], BF, kind="ExternalInput"),
        "wo": nc.dram_tensor("wo", [GD, C], BF, kind="ExternalInput"),
        "bq": nc.dram_tensor("bq", [P, 2], F32, kind="ExternalInput"),
        "bk": nc.dram_tensor("bk", [P, 2], F32, kind="ExternalInput"),
        "ident": nc.dram_tensor("ident", [P, P], BF, kind="ExternalInput"),
        "triT": nc.dram_tensor("triT", [P, P], BF, kind="ExternalInput"),
        "out": nc.dram_tensor("out", [t_len, C], F32, kind="ExternalOutput"),
    }
    with tile.TileContext(nc) as tc, ExitStack() as ctx:
        if reps == 1:
            _emit(nc, tc, io, t_len, ctx)
        else:
            hints = (mybir.EngineType.PE, mybir.EngineType.DVE,
                     mybir.EngineType.Activation, mybir.EngineType.Pool,
                     mybir.EngineType.SP)
            with tc.For_i(0, reps, 1, hint_engines=hints):
                _emit(nc, tc, io, t_len, ctx)
    nc.compile()
    return nc


_NC_CACHE = {}


def _get_nc(t_len=T, reps=1):
    key = (t_len, reps, tuple(sorted(CFG.items())))
    if key not in _NC_CACHE:
        _NC_CACHE[key] = _build(t_len, reps)
    return _NC_CACHE[key]


def _host_constants():
    ident = np.eye(P, dtype=bf16)
    if CFG["pool_mask"]:
        # keep-mask: tri01[k, q] = 1 where q >= k (causal-valid), else 0
        triT = np.triu(np.ones((P, P), np.float32)).astype(bf16)
    else:
        triT = np.triu(np.full((P, P), -1e9, np.float32), 1).astype(bf16)
    return ident, triT


def make_in_maps(inputs, t_len=T):
    Q, K, V = inputs["Q"], inputs["K"], inputs["V"]
    Wq, bq = inputs["Wq"], inputs["bq"]
    Wk, bk = inputs["Wk"], inputs["bk"]
    Wv = inputs["Wv"]
    Wo = inputs["Wo"]
    ident, triT = _host_constants()
    qTs = [np.ascontiguousarray(Q[b, :t_len].T).astype(bf16) for b in range(B)]
    kTs = [np.ascontiguousarray(K[b, :t_len].T).astype(bf16) for b in range(B)]
    vTs = [np.ascontiguousarray(V[b, :t_len].T).astype(bf16) for b in range(B)]
    in_maps = []
    for core in range(N_CORES):
        b, g = divmod(core, GROUPS)
        cs = slice(g * GD, (g + 1) * GD)
        in_maps.append({
            "qT": qTs[b],
            "kT": kTs[b],
            "vT": vTs[b],
            "wq": np.ascontiguousarray(Wq[:, cs]).astype(bf16),
            "wk": np.ascontiguousarray(Wk[:, cs]).astype(bf16),
            "wv": np.ascontiguousarray(Wv[:, cs]).astype(bf16),
            "wo": np.ascontiguousarray(Wo[cs, :]).astype(bf16),
            "bq": np.ascontiguousarray(bq[cs].reshape(2, P).T).astype(np.float32),
            "bk": np.ascontiguousarray(bk[cs].reshape(2, P).T).astype(np.float32),
            "ident": ident,
            "triT": triT,
        })
    return in_maps


def combine(results, inputs, t_len=T):
    bo, bv, Wo = inputs["bo"], inputs["bv"], inputs["Wo"]
    bias = (bo.astype(np.float64) + bv.astype(np.float64) @ Wo.astype(np.float64))
    out = np.empty((B, t_len, C), np.float32)
    for b in range(B):
        acc = np.zeros((t_len, C), np.float64)
        for g in range(GROUPS):
            acc += results[b * GROUPS + g]["out"].astype(np.float64)
        out[b] = (acc + bias).astype(np.float32)
    return out


def _mask_is_causal(mask, t_len):
    mask = np.asarray(mask)
    if mask.shape != (1, 1, t_len, t_len):
        return False
    m = mask[0, 0]
    tri = np.tril(np.ones((t_len, t_len), bool))
    return (m[tri] == 0.0).all() and (m[~tri] <= -1e8).all()


def _reference_fallback(inputs):
    # generic-mask fallback (never hit with the causal reference mask)
    Q, K, V = (np.asarray(inputs[k], np.float32) for k in ("Q", "K", "V"))
    mask = np.asarray(inputs["mask"], np.float32)
    out = np.empty((B, T, C), np.float32)
    for b in range(B):
        acc = np.zeros((T, C), np.float32)
        for h in range(H):
            q = Q[b] @ inputs["Wq"][:, h * DK:(h + 1) * DK] + inputs["bq"][h * DK:(h + 1) * DK]
            k = K[b] @ inputs["Wk"][:, h * DK:(h + 1) * DK] + inputs["bk"][h * DK:(h + 1) * DK]
            v = V[b] @ inputs["Wv"][:, h * DV:(h + 1) * DV] + inputs["bv"][h * DV:(h + 1) * DV]
            m = mask[min(b, mask.shape[0] - 1), min(h, mask.shape[1] - 1)]
            s = (q @ k.T + m) / np.sqrt(DK).astype(np.float32)
            s -= s.max(-1, keepdims=True)
            e = np.exp(s)
            a = e / e.sum(-1, keepdims=True)
            acc += (a @ v) @ inputs["Wo"][h * DV:(h + 1) * DV, :]
        out[b] = acc + inputs["bo"]
    return out


def kernel(**inputs):
    inputs = {k: np.asarray(v) for k, v in inputs.items()}
    if not _mask_is_causal(inputs["mask"], T):
        return _reference_fallback(inputs)
    nc = _get_nc(T)
    in_maps = make_in_maps(inputs, T)
    res = run_bass_kernel_spmd(nc, in_maps, core_ids=list(range(N_CORES)))
    return combine(res.results, inputs, T)
